# revision 1
# baseline (speedup 1.0000x reference)
"""Periodic-kernel attention on 8 TRN2 NeuronCores (v2).

Math (per head h):
  qn = q/|q|, kn = k/|k|, cos = qn.kn
  pre = (cos(2*pi*sqrt(2-2*cos)) - 1)/8 + (|q|^2 + |k|^2)/16
  out = softmax_k(pre) @ v

Let u = (1-cos)/2, z = cos(2*pi*sqrt(u))/2. Then the periodic part of the
exponent is exactly z^2 - 1/4, so softmax weights are proportional to
exp(z^2) (constants cancel; the |k|^2 term is a per-key scale g applied
host-side, |q|^2 cancels in softmax).

Device chain per 128x512 score tile (24 shards = 12 heads x 2 query-halves,
3 per core):
  x = alpha*u + beta via one fp16 PE matmul with extended 66-dim Q/K vectors
  s = z^2 via one custom 8-op DVE pass:  y=x^2+C0; v=(y^2+C1)*y; s=(v^2-.5)^2
  e = exp(s) via one ACT pass (fp16 out)
  av += WV @ e accumulated on PE, WV = [V*g | g] so the softmax denominator
  is the last accumulator row; the divide happens host-side after gather.
"""

import sys

if "/opt/trn_rl_repo" not in sys.path:
    sys.path.insert(0, "/opt/trn_rl_repo")

import numpy as np

import concourse.bacc as bacc
import concourse.bass as bass
import concourse.mybir as mybir
import concourse.tile as tile
from concourse import bass_utils, dve_ops
from concourse.dve_spec import C0, C1, C2, Spec, Src0, _has_src1, lower, sq
from concourse.dve_uop import DveOpSpec

H, S, D = 12, 2048, 64
NCORES = 8
M_PER = 3  # shards per core (24 / 8)
QH = S // 2  # queries per shard
KC = 16  # key chunks of 128
EXK = D + 2  # score contraction dim: 64 + two const columns
EXV = D + 1  # wv columns: 64 vals + denominator

# minimax fit of z = cos(2*pi*sqrt(u))/2 on u in [0,1] for the 8-op body
AL = 0.27692346002555385
BE = -1.5703144799204443
PC0 = -0.8784734114616589
PC1 = -1.889973842139018

f32 = np.float32
f16 = np.float16


def _pkc2s_ref(in0, in1, c0, c1, c2):
    x = np.asarray(in0, dtype=f32)
    c0, c1, c2 = f32(c0), f32(c1), f32(c2)
    t1 = x * x
    y = t1 + c0
    t2 = y * y
    t3 = t2 + c1
    v = t3 * y
    t4 = v * v
    t5 = t4 - c2
    return t5 * t5


def _pkc2s_spec():
    y = sq(Src0) + C0
    v = (sq(y) + C1) * y
    return Spec(body=sq(sq(v) - C2), reference=_pkc2s_ref)


def _register_dve(name, spec):
    for op in dve_ops.OPS:
        if op.name == name:
            return op
    row = dve_ops._CUSTOM_DVE_ROW_BASE + len(dve_ops.OPS)
    assert row < 0x20, "custom-DVE row overflow"
    dve_ops._SUB_OPCODE_FOR_NAME[name] = row
    shas = {
        ver: DveOpSpec(
            name=name, opcode=row, uops=lower(spec, ver=ver), rd1_en=_has_src1(spec)
        ).sha(ver)
        for ver in ("v3", "v4")
    }
    op = dve_ops.DveOp(name=name, spec=spec, subdim=False, uops_sha=shas)
    dve_ops.OPS.append(op)
    dve_ops.CUSTOM_DVE_SPECS[name] = spec
    return op


def build_program():
    pk_op = _register_dve("PKC2S", _pkc2s_spec())

    nc = bacc.Bacc(
        "TRN2", target_bir_lowering=False, debug=False, num_devices=NCORES
    )
    kt_d = nc.dram_tensor("kt", (M_PER, EXK, S), mybir.dt.float16, kind="ExternalInput")
    qt_d = nc.dram_tensor(
        "qt", (M_PER, EXK, QH), mybir.dt.float16, kind="ExternalInput"
    )
    wv_d = nc.dram_tensor(
        "wv", (M_PER, 128, KC * EXV), mybir.dt.float16, kind="ExternalInput"
    )
    out_d = nc.dram_tensor(
        "out", (M_PER, 2, EXV, 512), mybir.dt.float32, kind="ExternalOutput"
    )
    # m=0 startup: small contiguous head tensors so the first matmul pair
    # (kt chunks 0-1 x first qcols) unblocks before the bulk transfers land
    kh_d = nc.dram_tensor("kh", (EXK, 256), mybir.dt.float16, kind="ExternalInput")
    qh_d = nc.dram_tensor("qh", (EXK, 512), mybir.dt.float16, kind="ExternalInput")
    kr_d = nc.dram_tensor("kr", (EXK, S - 256), mybir.dt.float16, kind="ExternalInput")
    qr_d = nc.dram_tensor("qr", (EXK, QH - 512), mybir.dt.float16, kind="ExternalInput")

    FP32, FP16 = mybir.dt.float32, mybir.dt.float16
    with tile.TileContext(nc) as tc:
        with (
            tc.tile_pool(name="inp", bufs=2) as inp_pool,
            tc.tile_pool(name="sbe", bufs=3) as s_pool,
            tc.tile_pool(name="ebe", bufs=3) as e_pool,
            tc.tile_pool(name="osb", bufs=2) as o_pool,
            tc.tile_pool(name="ps_s", bufs=3, space=bass.MemorySpace.PSUM) as ps_s_pool,
            tc.tile_pool(name="ps_av", bufs=2, space=bass.MemorySpace.PSUM) as ps_av_pool,
        ):
            for m in range(M_PER):
                kt_sb = inp_pool.tile((EXK, S), FP16, tag="kt")
                qt_sb = inp_pool.tile((EXK, QH), FP16, tag="qt")
                wv_sb = inp_pool.tile((128, KC * EXV), FP16, tag="wv")
                if m == 0:
                    nc.sync.dma_start(kt_sb[:, :256], kh_d[:, :])
                    nc.sync.dma_start(qt_sb[:, :512], qh_d[:, :])
                    nc.sync.dma_start(kt_sb[:, 256:], kr_d[:, :])
                    nc.sync.dma_start(wv_sb, wv_d[m])
                    nc.sync.dma_start(qt_sb[:, 512:], qr_d[:, :])
                else:
                    nc.sync.dma_start(kt_sb, kt_d[m])
                    nc.sync.dma_start(qt_sb, qt_d[m])
                    nc.sync.dma_start(wv_sb, wv_d[m])

                for qs in range(2):
                    ps_av = ps_av_pool.tile((EXV, 512), FP32, tag="av")
                    qcols = qt_sb[:, qs * 512 : (qs + 1) * 512]
                    for a in range(4):
                        s32 = s_pool.tile((128, 2048), FP32, tag="s")
                        for dg in range(2):
                            ps_s = ps_s_pool.tile((128, 1024), FP32, tag="ps")
                            for t in range(2):
                                kc = a * 4 + dg * 2 + t
                                nc.tensor.matmul(
                                    ps_s[:, t * 512 : (t + 1) * 512],
                                    kt_sb[:, kc * 128 : (kc + 1) * 128],
                                    qcols,
                                    start=True,
                                    stop=True,
                                )
                            nc.vector._custom_dve(
                                pk_op,
                                out=s32[:, dg * 1024 : (dg + 1) * 1024],
                                in0=ps_s,
                                s0=PC0,
                                s1=PC1,
                                imm2=0.5,
                            )
                        e16 = e_pool.tile((128, 2048), FP16, tag="e")
                        nc.scalar.activation(
                            e16, s32, mybir.ActivationFunctionType.Exp, scale=1.0
                        )
                        for t in range(4):
                            kc = a * 4 + t
                            nc.tensor.matmul(
                                ps_av,
                                wv_sb[:, kc * EXV : (kc + 1) * EXV],
                                e16[:, t * 512 : (t + 1) * 512],
                                start=(kc == 0),
                                stop=(kc == KC - 1),
                            )
                    av_sb = o_pool.tile((EXV, 512), FP32, tag="osb")
                    nc.scalar.copy(av_sb, ps_av)
                    nc.sync.dma_start(out_d[m, qs], av_sb)

    return nc


_STATE = None


def _get_state():
    global _STATE
    if _STATE is None:
        nc = build_program()
        nc.finalize()
        _STATE = nc
    return _STATE


def _host_prep(query, keys, vals):
    q = np.asarray(query, dtype=np.float64)[0]  # [H,S,D]
    k = np.asarray(keys, dtype=np.float64)[0]
    v = np.asarray(vals, dtype=f32)[0]

    qn = q / np.linalg.norm(q, axis=-1, keepdims=True)
    kn = (k / np.linalg.norm(k, axis=-1, keepdims=True)).astype(f16)
    k_sq = np.sum(k * k, axis=-1)  # [H,S] f64
    g = np.exp(k_sq / 16.0 - k_sq.max(axis=-1, keepdims=True) / 16.0).astype(f32)

    WV = np.concatenate([v * g[:, :, None], g[:, :, None]], axis=-1).astype(f16)

    bp = AL / 2 + BE
    bp_hi = f16(bp)
    bp_lo = f16(bp - np.float64(bp_hi))
    QT = np.concatenate(
        [
            (f32(-AL / 2) * qn.astype(f32)).astype(f16),
            np.full((H, S, 1), bp_hi, f16),
            np.full((H, S, 1), bp_lo, f16),
        ],
        axis=-1,
    )  # [H,S,66]
    KT = np.concatenate([kn, np.ones((H, S, 2), f16)], axis=-1)  # [H,S,66]

    in_maps = []
    for c in range(NCORES):
        kt_c = np.empty((M_PER, EXK, S), f16)
        qt_c = np.empty((M_PER, EXK, QH), f16)
        wv_c = np.empty((M_PER, 128, KC * EXV), f16)
        for m in range(M_PER):
            sh = M_PER * c + m
            h, j = divmod(sh, 2)
            kt_c[m] = KT[h].T
            qt_c[m] = QT[h, j * QH : (j + 1) * QH].T
            wv_c[m] = (
                WV[h].reshape(KC, 128, EXV).transpose(1, 0, 2).reshape(128, KC * EXV)
            )
        in_maps.append(
            {
                "kt": np.ascontiguousarray(kt_c),
                "qt": np.ascontiguousarray(qt_c),
                "wv": np.ascontiguousarray(wv_c),
                "kh": np.ascontiguousarray(kt_c[0][:, :256]),
                "qh": np.ascontiguousarray(qt_c[0][:, :512]),
                "kr": np.ascontiguousarray(kt_c[0][:, 256:]),
                "qr": np.ascontiguousarray(qt_c[0][:, 512:]),
            }
        )
    return in_maps


def _gather(results):
    out = np.empty((1, H, S, D), f32)
    for c in range(NCORES):
        o = np.asarray(results[c]["out"], dtype=f32)  # [3,2,65,512]
        num = o[:, :, :D, :]  # [3,2,64,512]
        den = o[:, :, D, :]  # [3,2,512]
        res = (num / den[:, :, None, :]).transpose(0, 1, 3, 2)  # [3,2,512,64]
        for m in range(M_PER):
            sh = M_PER * c + m
            h, j = divmod(sh, 2)
            blk = res[m].reshape(QH, D)
            out[0, h, j * QH : (j + 1) * QH, :] = blk
    return out


def _run(inputs, trace=False, **trace_kwargs):
    nc = _get_state()
    in_maps = _host_prep(inputs["query"], inputs["keys"], inputs["vals"])
    res = bass_utils.run_bass_kernel_spmd(
        nc, in_maps, list(range(NCORES)), trace=trace, **trace_kwargs
    )
    return _gather(res.results), res.exec_time_ns


def kernel(**inputs):
    out, _ = _run(inputs)
    return out



# revision 2
# speedup vs baseline: 2.3012x; 2.3012x over previous
"""Periodic-kernel attention on 8 TRN2 NeuronCores (v3).

Math (per head h):
  qn = q/|q|, kn = k/|k|, cos = qn.kn
  pre = (cos(2*pi*sqrt(2-2*cos)) - 1)/8 + (|q|^2 + |k|^2)/16
  out = softmax_k(pre) @ v

Let u = (1-cos)/2, z = cos(2*pi*sqrt(u))/2. Then the periodic part of the
exponent is exactly z^2 - 1/4, so softmax weights are proportional to
exp(z^2) (constants cancel; the |k|^2 term is a per-key scale g applied
via WV = [V*g | g], |q|^2 cancels in softmax).

Device chain per 128x512 score tile (24 shards = 12 heads x 2 query-halves,
3 per core):
  x = alpha*u + beta via one fp16 PE matmul with extended 66-dim Q/K vectors
  s = z^2 via one custom 8-op DVE pass:  y=x^2+C0; v=(y^2+C1)*y; s=(v^2-.5)^2
  e = exp(s) via one ACT pass (fp16 out)
  av += WV @ e accumulated on PE; row 64 of the accumulator is the softmax
  denominator, divided out on device (DVE recip + PE partition-broadcast +
  DVE multiply) so only the final fp16 [64,512] tile ships back.

v3 changes vs v2 (the wall clock is dominated by the axon tunnel:
~0.1 s RTT per blocking round trip, ~90 MB/s each way):
  - the jitted PJRT dispatch is built once and cached (v2 re-traced
    jax.jit(shard_map(...)) on every call),
  - donated output buffers are created on-device (v2 uploaded 8.5 MB of
    zeros per call),
  - per-head K/WV tensors are stored once per core (2 planes) instead of
    per shard (3), and the v2 startup-split duplicates are gone:
    uplink 19.4 MB -> 11.0 MB,
  - softmax division happens on device and the output returns as fp16
    [64,512] tiles: downlink 8.5 MB -> 3.1 MB,
  - host prep runs in float32 (v2 used float64).
"""

import sys

if "/opt/trn_rl_repo" not in sys.path:
    sys.path.insert(0, "/opt/trn_rl_repo")

import numpy as np

import concourse.bacc as bacc
import concourse.bass as bass
import concourse.mybir as mybir
import concourse.tile as tile
from concourse import dve_ops
from concourse.dve_spec import C0, C1, C2, Spec, Src0, _has_src1, lower, sq
from concourse.dve_uop import DveOpSpec

H, S, D = 12, 2048, 64
NCORES = 8
M_PER = 3  # shards per core (24 / 8)
QH = S // 2  # queries per shard
KC = 16  # key chunks of 128
EXK = D + 2  # score contraction dim: 64 + two const columns
EXV = D + 1  # wv columns: 64 vals + denominator

# minimax fit of z = cos(2*pi*sqrt(u))/2 on u in [0,1] for the 8-op body
AL = 0.27692346002555385
BE = -1.5703144799204443
PC0 = -0.8784734114616589
PC1 = -1.889973842139018

f32 = np.float32
f16 = np.float16


def _pkc2s_ref(in0, in1, c0, c1, c2):
    x = np.asarray(in0, dtype=f32)
    c0, c1, c2 = f32(c0), f32(c1), f32(c2)
    t1 = x * x
    y = t1 + c0
    t2 = y * y
    t3 = t2 + c1
    v = t3 * y
    t4 = v * v
    t5 = t4 - c2
    return t5 * t5


def _pkc2s_spec():
    y = sq(Src0) + C0
    v = (sq(y) + C1) * y
    return Spec(body=sq(sq(v) - C2), reference=_pkc2s_ref)


def _register_dve(name, spec):
    for op in dve_ops.OPS:
        if op.name == name:
            return op
    row = dve_ops._CUSTOM_DVE_ROW_BASE + len(dve_ops.OPS)
    assert row < 0x20, "custom-DVE row overflow"
    dve_ops._SUB_OPCODE_FOR_NAME[name] = row
    shas = {
        ver: DveOpSpec(
            name=name, opcode=row, uops=lower(spec, ver=ver), rd1_en=_has_src1(spec)
        ).sha(ver)
        for ver in ("v3", "v4")
    }
    op = dve_ops.DveOp(name=name, spec=spec, subdim=False, uops_sha=shas)
    dve_ops.OPS.append(op)
    dve_ops.CUSTOM_DVE_SPECS[name] = spec
    return op


# Per-core head storage (2 planes) and shard map. Core c holds heads
# [HA_c, HB_c]; shard m in {0,1} is (HA, half m), shard 2 is
# (HB, half 0) on even cores / (HB, half 1) on odd cores. This covers all
# 24 (head, half) pairs exactly once with a core-independent device
# program (kt/wv plane per m is always [0, 0, 1]).
def _shard_map(c):
    if c % 2 == 0:
        ha = 3 * c // 2
        hb = ha + 1
        return (ha, hb), [(ha, 0), (ha, 1), (hb, 0)]
    hb = (3 * c - 1) // 2
    ha = hb + 1
    return (ha, hb), [(ha, 0), (ha, 1), (hb, 1)]


def build_program():
    pk_op = _register_dve("PKC2S", _pkc2s_spec())

    nc = bacc.Bacc(
        "TRN2", target_bir_lowering=False, debug=False, num_devices=NCORES
    )
    kt_d = nc.dram_tensor("kt", (2, EXK, S), mybir.dt.float16, kind="ExternalInput")
    qt_d = nc.dram_tensor(
        "qt", (M_PER, EXK, QH), mybir.dt.float16, kind="ExternalInput"
    )
    wv_d = nc.dram_tensor(
        "wv", (2, 128, KC * EXV), mybir.dt.float16, kind="ExternalInput"
    )
    out_d = nc.dram_tensor(
        "out", (M_PER, 2, D, 512), mybir.dt.float16, kind="ExternalOutput"
    )

    FP32, FP16 = mybir.dt.float32, mybir.dt.float16
    with tile.TileContext(nc) as tc:
        with (
            tc.tile_pool(name="inp", bufs=1) as inp_pool,
            tc.tile_pool(name="sbe", bufs=3) as s_pool,
            tc.tile_pool(name="ebe", bufs=3) as e_pool,
            tc.tile_pool(name="osb", bufs=2) as o_pool,
            tc.tile_pool(name="ps_s", bufs=3, space=bass.MemorySpace.PSUM) as ps_s_pool,
            tc.tile_pool(name="ps_av", bufs=1, space=bass.MemorySpace.PSUM) as ps_av_pool,
            tc.tile_pool(name="ps_b", bufs=1, space=bass.MemorySpace.PSUM) as ps_b_pool,
        ):
            ones_sb = inp_pool.tile((1, D), FP32, tag="ones")
            nc.vector.memset(ones_sb, 1.0)

            kt_sb = []
            wv_sb = []
            qt_sb = []
            for p in range(2):
                t_kt = inp_pool.tile((EXK, S), FP16, tag=f"kt{p}")
                nc.sync.dma_start(t_kt, kt_d[p])
                kt_sb.append(t_kt)
                t_wv = inp_pool.tile((128, KC * EXV), FP16, tag=f"wv{p}")
                nc.sync.dma_start(t_wv, wv_d[p])
                wv_sb.append(t_wv)
            for m in range(M_PER):
                t_qt = inp_pool.tile((EXK, QH), FP16, tag=f"qt{m}")
                nc.sync.dma_start(t_qt, qt_d[m])
                qt_sb.append(t_qt)

            for m in range(M_PER):
                lh = 0 if m < 2 else 1
                for qs in range(2):
                    ps_av = ps_av_pool.tile((EXV, 512), FP32, tag="av")
                    qcols = qt_sb[m][:, qs * 512 : (qs + 1) * 512]
                    for a in range(4):
                        s32 = s_pool.tile((128, 2048), FP32, tag="s")
                        for dg in range(2):
                            ps_s = ps_s_pool.tile((128, 1024), FP32, tag="ps")
                            for t in range(2):
                                kc = a * 4 + dg * 2 + t
                                nc.tensor.matmul(
                                    ps_s[:, t * 512 : (t + 1) * 512],
                                    kt_sb[lh][:, kc * 128 : (kc + 1) * 128],
                                    qcols,
                                    start=True,
                                    stop=True,
                                )
                            nc.vector._custom_dve(
                                pk_op,
                                out=s32[:, dg * 1024 : (dg + 1) * 1024],
                                in0=ps_s,
                                s0=PC0,
                                s1=PC1,
                                imm2=0.5,
                            )
                        e16 = e_pool.tile((128, 2048), FP16, tag="e")
                        nc.scalar.activation(
                            e16, s32, mybir.ActivationFunctionType.Exp, scale=1.0
                        )
                        for t in range(4):
                            kc = a * 4 + t
                            nc.tensor.matmul(
                                ps_av,
                                wv_sb[lh][:, kc * EXV : (kc + 1) * EXV],
                                e16[:, t * 512 : (t + 1) * 512],
                                start=(kc == 0),
                                stop=(kc == KC - 1),
                            )
                    av_sb = o_pool.tile((EXV, 512), FP32, tag="avsb")
                    nc.scalar.copy(av_sb, ps_av)
                    rcp = o_pool.tile((1, 512), FP32, tag="rcp")
                    nc.vector.reciprocal(rcp, av_sb[D : D + 1, :])
                    ps_b = ps_b_pool.tile((D, 512), FP32, tag="b")
                    nc.tensor.matmul(ps_b, ones_sb, rcp, start=True, stop=True)
                    out16 = o_pool.tile((D, 512), FP16, tag="o16")
                    nc.vector.tensor_tensor(
                        out16, av_sb[:D, :], ps_b, mybir.AluOpType.mult
                    )
                    nc.sync.dma_start(out_d[m, qs], out16)

    return nc


_STATE = None


def _build_exec():
    import jax
    import jax.numpy as jnp
    from jax.experimental.shard_map import shard_map
    from jax.sharding import Mesh, NamedSharding, PartitionSpec

    from concourse.bass2jax import (
        _bass_exec_p,
        install_neuronx_cc_hook,
        partition_id_tensor,
    )

    nc = build_program()
    nc.finalize()
    install_neuronx_cc_hook()
    assert nc.dbg_addr is None

    partition_name = nc.partition_id_tensor.name if nc.partition_id_tensor else None
    in_names, out_names, out_avals = [], [], []
    for alloc in nc.m.functions[0].allocations:
        if not isinstance(alloc, mybir.MemoryLocationSet):
            continue
        name = alloc.memorylocations[0].name
        if alloc.kind == "ExternalInput":
            if name != partition_name:
                in_names.append(name)
        elif alloc.kind == "ExternalOutput":
            out_names.append(name)
            out_avals.append(
                jax.core.ShapedArray(
                    tuple(alloc.tensor_shape), mybir.dt.np(alloc.dtype)
                )
            )
    n_params = len(in_names)
    n_outs = len(out_avals)
    all_in_names = list(in_names) + list(out_names)
    if partition_name is not None:
        all_in_names.append(partition_name)
    donate = tuple(range(n_params, n_params + n_outs))

    def _body(*args):
        operands = list(args)
        if partition_name is not None:
            operands.append(partition_id_tensor())
        return tuple(
            _bass_exec_p.bind(
                *operands,
                out_avals=tuple(out_avals),
                in_names=tuple(all_in_names),
                out_names=tuple(out_names),
                lowering_input_output_aliases=(),
                sim_require_finite=True,
                sim_require_nnan=True,
                nc=nc,
            )
        )

    devices = jax.devices()[:NCORES]
    assert len(devices) == NCORES
    mesh = Mesh(np.asarray(devices), ("core",))
    sh = NamedSharding(mesh, PartitionSpec("core"))
    sharded = jax.jit(
        shard_map(
            _body,
            mesh=mesh,
            in_specs=(PartitionSpec("core"),) * (n_params + n_outs),
            out_specs=(PartitionSpec("core"),) * n_outs,
            check_rep=False,
        ),
        donate_argnums=donate,
        keep_unused=True,
    )
    zero_shapes = [
        (NCORES * av.shape[0], *av.shape[1:]) for av in out_avals
    ]
    zero_dtypes = [av.dtype for av in out_avals]
    zeros_fn = jax.jit(
        lambda: tuple(
            jnp.zeros(s, d) for s, d in zip(zero_shapes, zero_dtypes)
        ),
        out_shardings=(sh,) * n_outs,
    )
    return nc, sharded, zeros_fn, in_names, out_names, out_avals


def _get_exec():
    global _STATE
    if _STATE is None:
        _STATE = _build_exec()
    return _STATE


def _get_state():
    return _get_exec()[0]


def _host_prep(query, keys, vals):
    q = np.asarray(query, dtype=f32).reshape(H, S, D)
    k = np.asarray(keys, dtype=f32).reshape(H, S, D)
    v = np.asarray(vals, dtype=f32).reshape(H, S, D)

    k_sq = np.einsum("hsd,hsd->hs", k, k)
    kn = k * (1.0 / np.sqrt(k_sq))[:, :, None]
    g = np.exp((k_sq - k_sq.max(axis=-1, keepdims=True)) / 16.0)
    q_sq = np.einsum("hsd,hsd->hs", q, q)
    qsc = q * (f32(-AL / 2) / np.sqrt(q_sq))[:, :, None]

    bp = AL / 2 + BE
    bp_hi = f16(bp)
    bp_lo = f16(np.float64(bp) - np.float64(bp_hi))

    KT = np.empty((H, EXK, S), f16)
    KT[:, :D, :] = kn.transpose(0, 2, 1)
    KT[:, D:, :] = 1.0
    QT = np.empty((H, EXK, S), f16)
    QT[:, :D, :] = qsc.transpose(0, 2, 1)
    QT[:, D, :] = bp_hi
    QT[:, D + 1, :] = bp_lo
    WV = np.empty((H, S, EXV), f16)
    WV[:, :, :D] = v * g[:, :, None]
    WV[:, :, D] = g
    WVp = (
        WV.reshape(H, KC, 128, EXV)
        .transpose(0, 2, 1, 3)
        .reshape(H, 128, KC * EXV)
    )

    head_order = []
    qt_all = np.empty((NCORES * M_PER, EXK, QH), f16)
    for c in range(NCORES):
        (ha, hb), shards = _shard_map(c)
        head_order += [ha, hb]
        for m, (h, j) in enumerate(shards):
            qt_all[M_PER * c + m] = QT[h, :, j * QH : (j + 1) * QH]
    kt_all = KT[head_order]
    wv_all = WVp[head_order]
    return [kt_all, qt_all, wv_all]


def _gather(out_g):
    # out_g: [NCORES*M_PER, 2, D, 512] fp16 (already divided by the softmax
    # denominator on device)
    out = np.empty((1, H, S, D), f32)
    for c in range(NCORES):
        _, shards = _shard_map(c)
        for m, (h, j) in enumerate(shards):
            blk = out_g[M_PER * c + m]  # [2, D, 512]
            base = j * QH
            for qs in range(2):
                out[0, h, base + qs * 512 : base + (qs + 1) * 512, :] = blk[qs].T
    return out


def _run(inputs, trace=False, **trace_kwargs):
    import jax

    nc, sharded, zeros_fn, in_names, out_names, out_avals = _get_exec()
    if trace:
        # fall back to the library path for tracing (slow, but only used
        # for profiling runs)
        from concourse import bass_utils

        kt_all, qt_all, wv_all = _host_prep(
            inputs["query"], inputs["keys"], inputs["vals"]
        )
        in_maps = [
            {
                "kt": kt_all[2 * c : 2 * c + 2],
                "qt": qt_all[M_PER * c : M_PER * (c + 1)],
                "wv": wv_all[2 * c : 2 * c + 2],
            }
            for c in range(NCORES)
        ]
        res = bass_utils.run_bass_kernel_spmd(
            nc, in_maps, list(range(NCORES)), trace=True, **trace_kwargs
        )
        out_g = np.concatenate(
            [np.asarray(res.results[c]["out"])[None] for c in range(NCORES)], 0
        ).reshape(NCORES * M_PER, 2, D, 512)
        return _gather(out_g), res.exec_time_ns

    prepped = _host_prep(inputs["query"], inputs["keys"], inputs["vals"])
    by_name = dict(zip(["kt", "qt", "wv"], prepped))
    concat_in = [by_name[name] for name in in_names]
    zeros = zeros_fn()
    out_arrs = sharded(*concat_in, *zeros)
    out_g = np.asarray(out_arrs[0]).reshape(NCORES * M_PER, 2, D, 512)
    return _gather(out_g), None


def kernel(**inputs):
    out, _ = _run(inputs)
    return out


# revision 12
# speedup vs baseline: 2.5386x; 1.1032x over previous
"""Periodic-kernel attention on 8 TRN2 NeuronCores (v3).

Math (per head h):
  qn = q/|q|, kn = k/|k|, cos = qn.kn
  pre = (cos(2*pi*sqrt(2-2*cos)) - 1)/8 + (|q|^2 + |k|^2)/16
  out = softmax_k(pre) @ v

Let u = (1-cos)/2, z = cos(2*pi*sqrt(u))/2. Then the periodic part of the
exponent is exactly z^2 - 1/4, so softmax weights are proportional to
exp(z^2) (constants cancel; the |k|^2 term is a per-key scale g applied
via WV = [V*g | g], |q|^2 cancels in softmax).

Device chain per 128x512 score tile (24 shards = 12 heads x 2 query-halves,
3 per core):
  x = alpha*u + beta via one fp16 PE matmul with extended 66-dim Q/K vectors
  s = z^2 via one custom 8-op DVE pass:  y=x^2+C0; v=(y^2+C1)*y; s=(v^2-.5)^2
  e = exp(s) via one ACT pass (fp16 out)
  av += WV @ e accumulated on PE; row 64 of the accumulator is the softmax
  denominator, divided out on device (DVE recip + PE partition-broadcast +
  DVE multiply) so only the final fp16 [64,512] tile ships back.

v3 changes vs v2 (the wall clock is dominated by the axon tunnel:
~0.1 s RTT per blocking round trip, ~90 MB/s each way):
  - the jitted PJRT dispatch is built once and cached (v2 re-traced
    jax.jit(shard_map(...)) on every call),
  - donated output buffers are created on-device (v2 uploaded 8.5 MB of
    zeros per call),
  - per-head K/WV tensors are stored once per core (2 planes) instead of
    per shard (3), and the v2 startup-split duplicates are gone:
    uplink 19.4 MB -> 11.0 MB,
  - softmax division happens on device and the output returns as fp16
    [64,512] tiles: downlink 8.5 MB -> 3.1 MB,
  - host prep runs in float32 (v2 used float64).
"""

import sys

if "/opt/trn_rl_repo" not in sys.path:
    sys.path.insert(0, "/opt/trn_rl_repo")

import numpy as np

import concourse.bacc as bacc
import concourse.bass as bass
import concourse.mybir as mybir
import concourse.tile as tile
from concourse import dve_ops
from concourse.dve_spec import C0, C1, C2, Spec, Src0, _has_src1, lower, sq
from concourse.dve_uop import DveOpSpec

f32 = np.float32
f16 = np.float16

H, S, D = 12, 2048, 64
NCORES = 8
M_PER = 3  # shards per core (24 / 8)
QH = S // 2  # queries per shard
KC = 16  # key chunks of 128
EXK = D + 1  # score contraction dim: 64 + one const column (bias row)
EXV = D + 1  # wv columns: 64 vals + denominator

# minimax fit of z = cos(2*pi*sqrt(u))/2 on u in [0,1] for the 8-op body
AL = 0.27692346002555385
BE = -1.5703144799204443
PC0 = -0.8784734114616589
PC1 = -1.889973842139018

# x = AL*u + BP reaches the PE as qsc.kn + f16(BP); the f16 rounding of the
# bias row is a known constant dx, folded into the polynomial's f32 C0
# immediate to first order around the midpoint x_bar = BP
BP = AL / 2 + BE
BP_HI = float(f16(BP))
_DX = BP - BP_HI
PC0C = PC0 + 2.0 * BP * _DX - _DX * _DX


def _pkc2s_ref(in0, in1, c0, c1, c2):
    x = np.asarray(in0, dtype=f32)
    c0, c1, c2 = f32(c0), f32(c1), f32(c2)
    t1 = x * x
    y = t1 + c0
    t2 = y * y
    t3 = t2 + c1
    v = t3 * y
    t4 = v * v
    t5 = t4 - c2
    return t5 * t5


def _pkc2s_spec():
    y = sq(Src0) + C0
    v = (sq(y) + C1) * y
    return Spec(body=sq(sq(v) - C2), reference=_pkc2s_ref)


def _register_dve(name, spec):
    for op in dve_ops.OPS:
        if op.name == name:
            return op
    row = dve_ops._CUSTOM_DVE_ROW_BASE + len(dve_ops.OPS)
    assert row < 0x20, "custom-DVE row overflow"
    dve_ops._SUB_OPCODE_FOR_NAME[name] = row
    shas = {
        ver: DveOpSpec(
            name=name, opcode=row, uops=lower(spec, ver=ver), rd1_en=_has_src1(spec)
        ).sha(ver)
        for ver in ("v3", "v4")
    }
    op = dve_ops.DveOp(name=name, spec=spec, subdim=False, uops_sha=shas)
    dve_ops.OPS.append(op)
    dve_ops.CUSTOM_DVE_SPECS[name] = spec
    return op


# Per-core head storage (2 planes) and shard map. Core c holds heads
# [HA_c, HB_c]; shard m in {0,1} is (HA, half m), shard 2 is
# (HB, half 0) on even cores / (HB, half 1) on odd cores. This covers all
# 24 (head, half) pairs exactly once with a core-independent device
# program (kt/wv plane per m is always [0, 0, 1]).
def _shard_map(c):
    if c % 2 == 0:
        ha = 3 * c // 2
        hb = ha + 1
        return (ha, hb), [(ha, 0), (ha, 1), (hb, 0)]
    hb = (3 * c - 1) // 2
    ha = hb + 1
    return (ha, hb), [(ha, 0), (ha, 1), (hb, 1)]


def build_program():
    pk_op = _register_dve("PKC2S", _pkc2s_spec())

    nc = bacc.Bacc(
        "TRN2", target_bir_lowering=False, debug=False, num_devices=NCORES
    )
    # const rows (ones for kt, bp_hi/bp_lo for qt) are memset on device, so
    # only the D=64 data rows are uploaded
    kt_d = nc.dram_tensor("kt", (2, D, S), mybir.dt.float16, kind="ExternalInput")
    qt_d = nc.dram_tensor(
        "qt", (M_PER, D, QH), mybir.dt.float16, kind="ExternalInput"
    )
    wv_d = nc.dram_tensor(
        "wv", (2, 128, KC * EXV), mybir.dt.float16, kind="ExternalInput"
    )
    out_d = nc.dram_tensor(
        "out", (M_PER, 2, D, 512), mybir.dt.float16, kind="ExternalOutput"
    )

    FP32, FP16 = mybir.dt.float32, mybir.dt.float16
    with tile.TileContext(nc) as tc:
        with (
            tc.tile_pool(name="inp", bufs=1) as inp_pool,
            tc.tile_pool(name="sbe", bufs=3) as s_pool,
            tc.tile_pool(name="ebe", bufs=3) as e_pool,
            tc.tile_pool(name="osb", bufs=2) as o_pool,
            tc.tile_pool(name="ps_s", bufs=3, space=bass.MemorySpace.PSUM) as ps_s_pool,
            tc.tile_pool(name="ps_av", bufs=1, space=bass.MemorySpace.PSUM) as ps_av_pool,
            tc.tile_pool(name="ps_b", bufs=1, space=bass.MemorySpace.PSUM) as ps_b_pool,
        ):
            ones_sb = inp_pool.tile((1, D), FP32, tag="ones")
            nc.vector.memset(ones_sb, 1.0)

            kt_sb = []
            wv_sb = []
            qt_sb = []
            for p in range(2):
                t_kt = inp_pool.tile((EXK, S), FP16, tag=f"kt{p}")
                nc.sync.dma_start(t_kt[:D, :], kt_d[p])
                nc.vector.memset(t_kt[D : D + 1, :], 1.0)
                kt_sb.append(t_kt)
                t_wv = inp_pool.tile((128, KC * EXV), FP16, tag=f"wv{p}")
                nc.sync.dma_start(t_wv, wv_d[p])
                wv_sb.append(t_wv)
            for m in range(M_PER):
                t_qt = inp_pool.tile((EXK, QH), FP16, tag=f"qt{m}")
                nc.sync.dma_start(t_qt[:D, :], qt_d[m])
                nc.vector.memset(t_qt[D : D + 1, :], BP_HI)
                qt_sb.append(t_qt)

            for m in range(M_PER):
                lh = 0 if m < 2 else 1
                for qs in range(2):
                    ps_av = ps_av_pool.tile((EXV, 512), FP32, tag="av")
                    qcols = qt_sb[m][:, qs * 512 : (qs + 1) * 512]
                    for a in range(4):
                        s32 = s_pool.tile((128, 2048), FP32, tag="s")
                        for dg in range(2):
                            ps_s = ps_s_pool.tile((128, 1024), FP32, tag="ps")
                            for t in range(2):
                                kc = a * 4 + dg * 2 + t
                                nc.tensor.matmul(
                                    ps_s[:, t * 512 : (t + 1) * 512],
                                    kt_sb[lh][:, kc * 128 : (kc + 1) * 128],
                                    qcols,
                                    start=True,
                                    stop=True,
                                )
                            nc.vector._custom_dve(
                                pk_op,
                                out=s32[:, dg * 1024 : (dg + 1) * 1024],
                                in0=ps_s,
                                s0=PC0C,
                                s1=PC1,
                                imm2=0.5,
                            )
                        e16 = e_pool.tile((128, 2048), FP16, tag="e")
                        nc.scalar.activation(
                            e16, s32, mybir.ActivationFunctionType.Exp, scale=1.0
                        )
                        for t in range(4):
                            kc = a * 4 + t
                            nc.tensor.matmul(
                                ps_av,
                                wv_sb[lh][:, kc * EXV : (kc + 1) * EXV],
                                e16[:, t * 512 : (t + 1) * 512],
                                start=(kc == 0),
                                stop=(kc == KC - 1),
                            )
                    av_sb = o_pool.tile((EXV, 512), FP32, tag="avsb")
                    nc.scalar.copy(av_sb, ps_av)
                    rcp = o_pool.tile((1, 512), FP32, tag="rcp")
                    nc.vector.reciprocal(rcp, av_sb[D : D + 1, :])
                    ps_b = ps_b_pool.tile((D, 512), FP32, tag="b")
                    nc.tensor.matmul(ps_b, ones_sb, rcp, start=True, stop=True)
                    out16 = o_pool.tile((D, 512), FP16, tag="o16")
                    nc.vector.tensor_tensor(
                        out16, av_sb[:D, :], ps_b, mybir.AluOpType.mult
                    )
                    nc.sync.dma_start(out_d[m, qs], out16)

    return nc


_STATE = None


def _build_exec():
    import jax
    import jax.numpy as jnp
    from jax.experimental.shard_map import shard_map
    from jax.sharding import Mesh, NamedSharding, PartitionSpec

    from concourse.bass2jax import (
        _bass_exec_p,
        install_neuronx_cc_hook,
        partition_id_tensor,
    )

    nc = build_program()
    nc.finalize()
    install_neuronx_cc_hook()
    assert nc.dbg_addr is None

    partition_name = nc.partition_id_tensor.name if nc.partition_id_tensor else None
    in_names, out_names, out_avals = [], [], []
    for alloc in nc.m.functions[0].allocations:
        if not isinstance(alloc, mybir.MemoryLocationSet):
            continue
        name = alloc.memorylocations[0].name
        if alloc.kind == "ExternalInput":
            if name != partition_name:
                in_names.append(name)
        elif alloc.kind == "ExternalOutput":
            out_names.append(name)
            out_avals.append(
                jax.core.ShapedArray(
                    tuple(alloc.tensor_shape), mybir.dt.np(alloc.dtype)
                )
            )
    n_params = len(in_names)
    n_outs = len(out_avals)
    all_in_names = list(in_names) + list(out_names)
    if partition_name is not None:
        all_in_names.append(partition_name)
    donate = tuple(range(n_params, n_params + n_outs))

    def _body(*args):
        operands = list(args)
        if partition_name is not None:
            operands.append(partition_id_tensor())
        return tuple(
            _bass_exec_p.bind(
                *operands,
                out_avals=tuple(out_avals),
                in_names=tuple(all_in_names),
                out_names=tuple(out_names),
                lowering_input_output_aliases=(),
                sim_require_finite=True,
                sim_require_nnan=True,
                nc=nc,
            )
        )

    devices = jax.devices()[:NCORES]
    assert len(devices) == NCORES
    mesh = Mesh(np.asarray(devices), ("core",))
    sh = NamedSharding(mesh, PartitionSpec("core"))
    sharded = jax.jit(
        shard_map(
            _body,
            mesh=mesh,
            in_specs=(PartitionSpec("core"),) * (n_params + n_outs),
            out_specs=(PartitionSpec("core"),) * n_outs,
            check_rep=False,
        ),
        donate_argnums=donate,
        keep_unused=True,
    )
    zero_shapes = [
        (NCORES * av.shape[0], *av.shape[1:]) for av in out_avals
    ]
    zero_dtypes = [av.dtype for av in out_avals]
    zeros_fn = jax.jit(
        lambda: tuple(
            jnp.zeros(s, d) for s, d in zip(zero_shapes, zero_dtypes)
        ),
        out_shardings=(sh,) * n_outs,
    )
    return nc, sharded, zeros_fn, in_names, out_names, sh


def _get_exec():
    global _STATE
    if _STATE is None:
        _STATE = _build_exec()
    return _STATE


def _get_state():
    return _get_exec()[0]


_HEAD_ORDER = [h for c in range(NCORES) for h in _shard_map(c)[0]]
_QT_SLOTS = [
    (M_PER * c + m, h, j)
    for c in range(NCORES)
    for m, (h, j) in enumerate(_shard_map(c)[1])
]


def _host_prep(query, keys, vals, put=None):
    """Build the three per-core global arrays. If `put` is given, each array
    is handed to it as soon as it is ready (async device upload overlapping
    the remaining prep) and the put results are returned instead."""
    q = np.asarray(query, dtype=f32).reshape(H, S, D)
    k = np.asarray(keys, dtype=f32).reshape(H, S, D)
    v = np.asarray(vals, dtype=f32).reshape(H, S, D)
    out = {}

    k_sq = np.einsum("hsd,hsd->hs", k, k)
    knT = (k * (1.0 / np.sqrt(k_sq))[:, :, None]).transpose(0, 2, 1)
    kt_all = np.empty((2 * NCORES, D, S), f16)
    for i, h in enumerate(_HEAD_ORDER):
        kt_all[i] = knT[h]
    out["kt"] = kt_all if put is None else put("kt", kt_all)

    g = np.exp((k_sq - k_sq.max(axis=-1, keepdims=True)) / 16.0)
    vg = v * g[:, :, None]
    wv_all = np.empty((2 * NCORES, 128, KC * EXV), f16)
    wv4 = wv_all.reshape(2 * NCORES, 128, KC, EXV)
    vg4 = vg.reshape(H, KC, 128, D)
    g3 = g.reshape(H, KC, 128)
    for i, h in enumerate(_HEAD_ORDER):
        wv4[i, :, :, :D] = vg4[h].transpose(1, 0, 2)
        wv4[i, :, :, D] = g3[h].T
    out["wv"] = wv_all if put is None else put("wv", wv_all)

    q_sq = np.einsum("hsd,hsd->hs", q, q)
    qscT = (q * (f32(-AL / 2) / np.sqrt(q_sq))[:, :, None]).transpose(0, 2, 1)
    qt_all = np.empty((NCORES * M_PER, D, QH), f16)
    for idx, h, j in _QT_SLOTS:
        qt_all[idx] = qscT[h][:, j * QH : (j + 1) * QH]
    out["qt"] = qt_all if put is None else put("qt", qt_all)
    return out


def _gather(out_g):
    # out_g: [NCORES*M_PER, 2, D, 512] fp16 (already divided by the softmax
    # denominator on device)
    out = np.empty((1, H, S, D), f32)
    for c in range(NCORES):
        _, shards = _shard_map(c)
        for m, (h, j) in enumerate(shards):
            blk = out_g[M_PER * c + m]  # [2, D, 512]
            base = j * QH
            for qs in range(2):
                out[0, h, base + qs * 512 : base + (qs + 1) * 512, :] = blk[qs].T
    return out


def _run(inputs, trace=False, **trace_kwargs):
    import jax

    nc, sharded, zeros_fn, in_names, out_names, sh = _get_exec()
    if trace:
        # fall back to the library path for tracing (slow, but only used
        # for profiling runs)
        from concourse import bass_utils

        arrs = _host_prep(inputs["query"], inputs["keys"], inputs["vals"])
        in_maps = [
            {
                "kt": arrs["kt"][2 * c : 2 * c + 2],
                "qt": arrs["qt"][M_PER * c : M_PER * (c + 1)],
                "wv": arrs["wv"][2 * c : 2 * c + 2],
            }
            for c in range(NCORES)
        ]
        res = bass_utils.run_bass_kernel_spmd(
            nc, in_maps, list(range(NCORES)), trace=True, **trace_kwargs
        )
        out_g = np.concatenate(
            [np.asarray(res.results[c]["out"])[None] for c in range(NCORES)], 0
        ).reshape(NCORES * M_PER, 2, D, 512)
        return _gather(out_g), res.exec_time_ns

    zeros = zeros_fn()
    dev = _host_prep(
        inputs["query"],
        inputs["keys"],
        inputs["vals"],
        put=lambda name, arr: jax.device_put(arr, sh),
    )
    out_arrs = sharded(*[dev[name] for name in in_names], *zeros)
    out_g = np.asarray(out_arrs[0]).reshape(NCORES * M_PER, 2, D, 512)
    return _gather(out_g), None


def kernel(**inputs):
    out, _ = _run(inputs)
    return out


# revision 21
# speedup vs baseline: 2.8035x; 1.1043x over previous
"""Periodic-kernel attention on 8 TRN2 NeuronCores (v3).

Math (per head h):
  qn = q/|q|, kn = k/|k|, cos = qn.kn
  pre = (cos(2*pi*sqrt(2-2*cos)) - 1)/8 + (|q|^2 + |k|^2)/16
  out = softmax_k(pre) @ v

Let u = (1-cos)/2, z = cos(2*pi*sqrt(u))/2. Then the periodic part of the
exponent is exactly z^2 - 1/4, so softmax weights are proportional to
exp(z^2) (constants cancel; the |k|^2 term is a per-key scale g applied
via WV = [V*g | g], |q|^2 cancels in softmax).

Device chain per 128x512 score tile (24 shards = 12 heads x 2 query-halves,
3 per core):
  x = alpha*u + beta via one fp16 PE matmul with extended 66-dim Q/K vectors
  s = z^2 via one custom 8-op DVE pass:  y=x^2+C0; v=(y^2+C1)*y; s=(v^2-.5)^2
  e = exp(s) via one ACT pass (fp16 out)
  av += WV @ e accumulated on PE; row 64 of the accumulator is the softmax
  denominator, divided out on device (DVE recip + PE partition-broadcast +
  DVE multiply) so only the final fp16 [64,512] tile ships back.

v3 changes vs v2 (the wall clock is dominated by the axon tunnel:
~0.1 s RTT per blocking round trip, ~90 MB/s each way):
  - the jitted PJRT dispatch is built once and cached (v2 re-traced
    jax.jit(shard_map(...)) on every call),
  - donated output buffers are created on-device (v2 uploaded 8.5 MB of
    zeros per call),
  - per-head K/WV tensors are stored once per core (2 planes) instead of
    per shard (3), and the v2 startup-split duplicates are gone:
    uplink 19.4 MB -> 11.0 MB,
  - softmax division happens on device and the output returns as fp16
    [64,512] tiles: downlink 8.5 MB -> 3.1 MB,
  - host prep runs in float32 (v2 used float64).
"""

import sys

if "/opt/trn_rl_repo" not in sys.path:
    sys.path.insert(0, "/opt/trn_rl_repo")

import numpy as np

import concourse.bacc as bacc
import concourse.bass as bass
import concourse.mybir as mybir
import concourse.tile as tile
from concourse import dve_ops
from concourse.dve_spec import C0, C1, C2, Spec, Src0, _has_src1, lower, sq
from concourse.dve_uop import DveOpSpec

f32 = np.float32
f16 = np.float16

H, S, D = 12, 2048, 64
NCORES = 8
M_PER = 3  # shards per core (24 / 8)
QH = S // 2  # queries per shard
KC = 16  # key chunks of 128
EXK = D + 1  # score contraction dim: 64 + one const column (bias row)
EXV = D + 1  # wv columns: 64 vals + denominator

# minimax fit of z = cos(2*pi*sqrt(u))/2 on u in [0,1] for the 8-op body
AL = 0.27692346002555385
BE = -1.5703144799204443
PC0 = -0.8784734114616589
PC1 = -1.889973842139018

# x = AL*u + BP reaches the PE as qsc.kn + f16(BP); the f16 rounding of the
# bias row is a known constant dx, folded into the polynomial's f32 C0
# immediate to first order around the midpoint x_bar = BP
BP = AL / 2 + BE
BP_HI = float(f16(BP))
_DX = BP - BP_HI
PC0C = PC0 + 2.0 * BP * _DX - _DX * _DX


def _pkc2s_ref(in0, in1, c0, c1, c2):
    x = np.asarray(in0, dtype=f32)
    c0, c1, c2 = f32(c0), f32(c1), f32(c2)
    t1 = x * x
    y = t1 + c0
    t2 = y * y
    t3 = t2 + c1
    v = t3 * y
    t4 = v * v
    t5 = t4 - c2
    return t5 * t5


def _pkc2s_spec():
    y = sq(Src0) + C0
    v = (sq(y) + C1) * y
    return Spec(body=sq(sq(v) - C2), reference=_pkc2s_ref)


def _register_dve(name, spec):
    for op in dve_ops.OPS:
        if op.name == name:
            return op
    row = dve_ops._CUSTOM_DVE_ROW_BASE + len(dve_ops.OPS)
    assert row < 0x20, "custom-DVE row overflow"
    dve_ops._SUB_OPCODE_FOR_NAME[name] = row
    shas = {
        ver: DveOpSpec(
            name=name, opcode=row, uops=lower(spec, ver=ver), rd1_en=_has_src1(spec)
        ).sha(ver)
        for ver in ("v3", "v4")
    }
    op = dve_ops.DveOp(name=name, spec=spec, subdim=False, uops_sha=shas)
    dve_ops.OPS.append(op)
    dve_ops.CUSTOM_DVE_SPECS[name] = spec
    return op


# Per-core head storage (2 planes) and shard map. Core c holds heads
# [HA_c, HB_c]; shard m in {0,1} is (HA, half m), shard 2 is
# (HB, half 0) on even cores / (HB, half 1) on odd cores. This covers all
# 24 (head, half) pairs exactly once with a core-independent device
# program (kt/wv plane per m is always [0, 0, 1]).
def _shard_map(c):
    if c % 2 == 0:
        ha = 3 * c // 2
        hb = ha + 1
        return (ha, hb), [(ha, 0), (ha, 1), (hb, 0)]
    hb = (3 * c - 1) // 2
    ha = hb + 1
    return (ha, hb), [(ha, 0), (ha, 1), (hb, 1)]


def build_program():
    pk_op = _register_dve("PKC2S", _pkc2s_spec())

    nc = bacc.Bacc(
        "TRN2", target_bir_lowering=False, debug=False, num_devices=NCORES
    )
    # const rows (ones for kt, bp_hi for qt) are memset on device, so only
    # the D=64 data rows are uploaded. Head A (2 query-half shards) is
    # uploaded in full; of the pair-shared head B each core uploads only its
    # half of the keys, and the halves are exchanged with the XOR-1
    # neighbor via an HBM AllGather over core pairs.
    kta_d = nc.dram_tensor("kta", (D, S), mybir.dt.float16, kind="ExternalInput")
    kth_d = nc.dram_tensor("kth", (D, QH), mybir.dt.float16, kind="ExternalInput")
    qt_d = nc.dram_tensor(
        "qt", (M_PER, D, QH), mybir.dt.float16, kind="ExternalInput"
    )
    wva_d = nc.dram_tensor(
        "wva", (128, KC * EXV), mybir.dt.float16, kind="ExternalInput"
    )
    wvh_d = nc.dram_tensor(
        "wvh", (128, KC * EXV // 2), mybir.dt.float16, kind="ExternalInput"
    )
    # collectives cannot read IO tensors, so the external half-tensors are
    # staged through internal DRAM copies first
    kthi_d = nc.dram_tensor("kthi", (D, QH), mybir.dt.float16, kind="Internal")
    wvhi_d = nc.dram_tensor(
        "wvhi", (128, KC * EXV // 2), mybir.dt.float16, kind="Internal"
    )
    ktg_d = nc.dram_tensor(
        "ktg", (2, D, QH), mybir.dt.float16, kind="Internal"
    )
    wvg_d = nc.dram_tensor(
        "wvg", (2, 128, KC * EXV // 2), mybir.dt.float16, kind="Internal"
    )
    out_d = nc.dram_tensor(
        "out", (M_PER, 2, D, 512), mybir.dt.float16, kind="ExternalOutput"
    )
    PAIRS = [[2 * k, 2 * k + 1] for k in range(NCORES // 2)]

    FP32, FP16 = mybir.dt.float32, mybir.dt.float16
    with tile.TileContext(nc) as tc:
        with (
            tc.tile_pool(name="inp", bufs=1) as inp_pool,
            tc.tile_pool(name="sbe", bufs=3) as s_pool,
            tc.tile_pool(name="ebe", bufs=3) as e_pool,
            tc.tile_pool(name="osb", bufs=2) as o_pool,
            tc.tile_pool(name="ps_s", bufs=3, space=bass.MemorySpace.PSUM) as ps_s_pool,
            tc.tile_pool(name="ps_av", bufs=1, space=bass.MemorySpace.PSUM) as ps_av_pool,
            tc.tile_pool(name="ps_b", bufs=1, space=bass.MemorySpace.PSUM) as ps_b_pool,
        ):
            ones_sb = inp_pool.tile((1, D), FP32, tag="ones")
            nc.vector.memset(ones_sb, 1.0)

            nc.sync.dma_start(kthi_d[:, :], kth_d[:, :])
            nc.sync.dma_start(wvhi_d[:, :], wvh_d[:, :])
            nc.gpsimd.collective_compute(
                "AllGather",
                mybir.AluOpType.bypass,
                PAIRS,
                [kthi_d[:, :]],
                [ktg_d[:, :, :]],
            )
            nc.gpsimd.collective_compute(
                "AllGather",
                mybir.AluOpType.bypass,
                PAIRS,
                [wvhi_d[:, :]],
                [wvg_d[:, :, :]],
            )

            kt_sb = []
            wv_sb = []
            qt_sb = []

            t_kt0 = inp_pool.tile((EXK, S), FP16, tag="kt0")
            nc.sync.dma_start(t_kt0[:D, :], kta_d[:, :])
            nc.vector.memset(t_kt0[D : D + 1, :], 1.0)
            kt_sb.append(t_kt0)
            t_wv0 = inp_pool.tile((128, KC * EXV), FP16, tag="wv0")
            nc.sync.dma_start(t_wv0, wva_d[:, :])
            wv_sb.append(t_wv0)

            t_kt1 = inp_pool.tile((EXK, S), FP16, tag="kt1")
            for gidx in range(2):
                nc.sync.dma_start(
                    t_kt1[:D, gidx * QH : (gidx + 1) * QH], ktg_d[gidx]
                )
            nc.vector.memset(t_kt1[D : D + 1, :], 1.0)
            kt_sb.append(t_kt1)
            t_wv1 = inp_pool.tile((128, KC * EXV), FP16, tag="wv1")
            half_w = KC * EXV // 2
            for gidx in range(2):
                nc.sync.dma_start(
                    t_wv1[:, gidx * half_w : (gidx + 1) * half_w], wvg_d[gidx]
                )
            wv_sb.append(t_wv1)

            for m in range(M_PER):
                t_qt = inp_pool.tile((EXK, QH), FP16, tag=f"qt{m}")
                nc.sync.dma_start(t_qt[:D, :], qt_d[m])
                nc.vector.memset(t_qt[D : D + 1, :], BP_HI)
                qt_sb.append(t_qt)

            for m in range(M_PER):
                lh = 0 if m < 2 else 1
                for qs in range(2):
                    ps_av = ps_av_pool.tile((EXV, 512), FP32, tag="av")
                    qcols = qt_sb[m][:, qs * 512 : (qs + 1) * 512]
                    for a in range(4):
                        s32 = s_pool.tile((128, 2048), FP32, tag="s")
                        for dg in range(2):
                            ps_s = ps_s_pool.tile((128, 1024), FP32, tag="ps")
                            for t in range(2):
                                kc = a * 4 + dg * 2 + t
                                nc.tensor.matmul(
                                    ps_s[:, t * 512 : (t + 1) * 512],
                                    kt_sb[lh][:, kc * 128 : (kc + 1) * 128],
                                    qcols,
                                    start=True,
                                    stop=True,
                                )
                            nc.vector._custom_dve(
                                pk_op,
                                out=s32[:, dg * 1024 : (dg + 1) * 1024],
                                in0=ps_s,
                                s0=PC0C,
                                s1=PC1,
                                imm2=0.5,
                            )
                        e16 = e_pool.tile((128, 2048), FP16, tag="e")
                        nc.scalar.activation(
                            e16, s32, mybir.ActivationFunctionType.Exp, scale=1.0
                        )
                        for t in range(4):
                            kc = a * 4 + t
                            nc.tensor.matmul(
                                ps_av,
                                wv_sb[lh][:, kc * EXV : (kc + 1) * EXV],
                                e16[:, t * 512 : (t + 1) * 512],
                                start=(kc == 0),
                                stop=(kc == KC - 1),
                            )
                    av_sb = o_pool.tile((EXV, 512), FP32, tag="avsb")
                    nc.scalar.copy(av_sb, ps_av)
                    rcp = o_pool.tile((1, 512), FP32, tag="rcp")
                    nc.vector.reciprocal(rcp, av_sb[D : D + 1, :])
                    ps_b = ps_b_pool.tile((D, 512), FP32, tag="b")
                    nc.tensor.matmul(ps_b, ones_sb, rcp, start=True, stop=True)
                    out16 = o_pool.tile((D, 512), FP16, tag="o16")
                    nc.vector.tensor_tensor(
                        out16, av_sb[:D, :], ps_b, mybir.AluOpType.mult
                    )
                    nc.sync.dma_start(out_d[m, qs], out16)

    return nc


_STATE = None


def _build_exec():
    import jax
    import jax.numpy as jnp
    from jax.experimental.shard_map import shard_map
    from jax.sharding import Mesh, NamedSharding, PartitionSpec

    from concourse.bass2jax import (
        _bass_exec_p,
        install_neuronx_cc_hook,
        partition_id_tensor,
    )

    nc = build_program()
    nc.finalize()
    install_neuronx_cc_hook()
    assert nc.dbg_addr is None

    partition_name = nc.partition_id_tensor.name if nc.partition_id_tensor else None
    in_names, out_names, out_avals = [], [], []
    for alloc in nc.m.functions[0].allocations:
        if not isinstance(alloc, mybir.MemoryLocationSet):
            continue
        name = alloc.memorylocations[0].name
        if alloc.kind == "ExternalInput":
            if name != partition_name:
                in_names.append(name)
        elif alloc.kind == "ExternalOutput":
            out_names.append(name)
            out_avals.append(
                jax.core.ShapedArray(
                    tuple(alloc.tensor_shape), mybir.dt.np(alloc.dtype)
                )
            )
    n_params = len(in_names)
    n_outs = len(out_avals)
    all_in_names = list(in_names) + list(out_names)
    if partition_name is not None:
        all_in_names.append(partition_name)
    donate = tuple(range(n_params, n_params + n_outs))

    def _body(*args):
        operands = list(args)
        if partition_name is not None:
            operands.append(partition_id_tensor())
        return tuple(
            _bass_exec_p.bind(
                *operands,
                out_avals=tuple(out_avals),
                in_names=tuple(all_in_names),
                out_names=tuple(out_names),
                lowering_input_output_aliases=(),
                sim_require_finite=True,
                sim_require_nnan=True,
                nc=nc,
            )
        )

    devices = jax.devices()[:NCORES]
    assert len(devices) == NCORES
    mesh = Mesh(np.asarray(devices), ("core",))
    sh = NamedSharding(mesh, PartitionSpec("core"))
    sharded = jax.jit(
        shard_map(
            _body,
            mesh=mesh,
            in_specs=(PartitionSpec("core"),) * (n_params + n_outs),
            out_specs=(PartitionSpec("core"),) * n_outs,
            check_rep=False,
        ),
        donate_argnums=donate,
        keep_unused=True,
    )
    zero_shapes = [
        (NCORES * av.shape[0], *av.shape[1:]) for av in out_avals
    ]
    zero_dtypes = [av.dtype for av in out_avals]
    zeros_fn = jax.jit(
        lambda: tuple(
            jnp.zeros(s, d) for s, d in zip(zero_shapes, zero_dtypes)
        ),
        out_shardings=(sh,) * n_outs,
    )
    return nc, sharded, zeros_fn, in_names, out_names, sh


def _get_exec():
    global _STATE
    if _STATE is None:
        _STATE = _build_exec()
    return _STATE


def _get_state():
    return _get_exec()[0]


_HEADS_A = [_shard_map(c)[0][0] for c in range(NCORES)]
_HEADS_B = [_shard_map(c)[0][1] for c in range(NCORES)]
_QT_SLOTS = [
    (M_PER * c + m, h, j)
    for c in range(NCORES)
    for m, (h, j) in enumerate(_shard_map(c)[1])
]


def _host_prep(query, keys, vals, put=None):
    """Build the per-core global arrays. If `put` is given, each array is
    handed to it as soon as it is ready (async device upload overlapping
    the remaining prep) and the put results are returned instead."""
    q = np.asarray(query, dtype=f32).reshape(H, S, D)
    k = np.asarray(keys, dtype=f32).reshape(H, S, D)
    v = np.asarray(vals, dtype=f32).reshape(H, S, D)
    out = {}

    def emit(name, arr):
        out[name] = arr if put is None else put(name, arr)

    k_sq = np.einsum("hsd,hsd->hs", k, k)
    knT = (k * (1.0 / np.sqrt(k_sq))[:, :, None]).transpose(0, 2, 1)
    kta = np.empty((NCORES, D, S), f16)
    kth = np.empty((NCORES, D, QH), f16)
    for c in range(NCORES):
        kta[c] = knT[_HEADS_A[c]]
        half = c % 2
        kth[c] = knT[_HEADS_B[c]][:, half * QH : (half + 1) * QH]
    # shard_map splits axis 0, so globals are concatenated along the
    # BIR-declared leading axis (a contiguous reshape here)
    emit("kta", kta.reshape(NCORES * D, S))
    emit("kth", kth.reshape(NCORES * D, QH))

    g = np.exp((k_sq - k_sq.max(axis=-1, keepdims=True)) / 16.0)
    vg = v * g[:, :, None]
    vg4 = vg.reshape(H, KC, 128, D)
    g3 = g.reshape(H, KC, 128)
    wva = np.empty((NCORES, 128, KC * EXV), f16)
    wva4 = wva.reshape(NCORES, 128, KC, EXV)
    wvh = np.empty((NCORES, 128, KC * EXV // 2), f16)
    wvh4 = wvh.reshape(NCORES, 128, KC // 2, EXV)
    for c in range(NCORES):
        ha, hb = _HEADS_A[c], _HEADS_B[c]
        wva4[c, :, :, :D] = vg4[ha].transpose(1, 0, 2)
        wva4[c, :, :, D] = g3[ha].T
        half = c % 2
        ksl = slice(half * KC // 2, (half + 1) * KC // 2)
        wvh4[c, :, :, :D] = vg4[hb, ksl].transpose(1, 0, 2)
        wvh4[c, :, :, D] = g3[hb, ksl].T
    emit("wva", wva.reshape(NCORES * 128, KC * EXV))
    emit("wvh", wvh.reshape(NCORES * 128, KC * EXV // 2))

    q_sq = np.einsum("hsd,hsd->hs", q, q)
    qscT = (q * (f32(-AL / 2) / np.sqrt(q_sq))[:, :, None]).transpose(0, 2, 1)
    qt_all = np.empty((NCORES * M_PER, D, QH), f16)
    for idx, h, j in _QT_SLOTS:
        qt_all[idx] = qscT[h][:, j * QH : (j + 1) * QH]
    emit("qt", qt_all)
    return out


def _gather(out_g):
    # out_g: [NCORES*M_PER, 2, D, 512] fp16 (already divided by the softmax
    # denominator on device)
    out = np.empty((1, H, S, D), f32)
    for c in range(NCORES):
        _, shards = _shard_map(c)
        for m, (h, j) in enumerate(shards):
            blk = out_g[M_PER * c + m]  # [2, D, 512]
            base = j * QH
            for qs in range(2):
                out[0, h, base + qs * 512 : base + (qs + 1) * 512, :] = blk[qs].T
    return out


def _run(inputs, trace=False, **trace_kwargs):
    import jax

    nc, sharded, zeros_fn, in_names, out_names, sh = _get_exec()
    if trace:
        # fall back to the library path for tracing (slow, but only used
        # for profiling runs)
        from concourse import bass_utils

        arrs = _host_prep(inputs["query"], inputs["keys"], inputs["vals"])
        in_maps = [
            {
                "kta": arrs["kta"].reshape(NCORES, D, S)[c],
                "kth": arrs["kth"].reshape(NCORES, D, QH)[c],
                "qt": arrs["qt"][M_PER * c : M_PER * (c + 1)],
                "wva": arrs["wva"].reshape(NCORES, 128, KC * EXV)[c],
                "wvh": arrs["wvh"].reshape(NCORES, 128, KC * EXV // 2)[c],
            }
            for c in range(NCORES)
        ]
        res = bass_utils.run_bass_kernel_spmd(
            nc, in_maps, list(range(NCORES)), trace=True, **trace_kwargs
        )
        out_g = np.concatenate(
            [np.asarray(res.results[c]["out"])[None] for c in range(NCORES)], 0
        ).reshape(NCORES * M_PER, 2, D, 512)
        return _gather(out_g), res.exec_time_ns

    zeros = zeros_fn()
    dev = _host_prep(
        inputs["query"],
        inputs["keys"],
        inputs["vals"],
        put=lambda name, arr: jax.device_put(arr, sh),
    )
    out_arrs = sharded(*[dev[name] for name in in_names], *zeros)
    out_g = np.asarray(out_arrs[0]).reshape(NCORES * M_PER, 2, D, 512)
    return _gather(out_g), None


def kernel(**inputs):
    out, _ = _run(inputs)
    return out
